# revision 77
# baseline (speedup 1.0000x reference)
"""Trainium2 Bass kernel for nn_GroupQueryAttention_51616916963669.

GQA with YaRN RoPE, sliding-window (128) + causal mask, learned sink logit,
qkv/out projections. B=1, S=2048, E=2048, H=32, G=8, D=64.

Sharding over 8 NeuronCores: 2-way sequence (1024 queries each, 128-token KV
halo) x 4-way heads (8 q-heads / 2 kv-groups per core). Each core computes a
partial out-projection over its 512 ctx dims; the host sums 4 head-partials
per sequence half and concatenates.

v3 design: scores computed TRANSPOSED (scT[k, q]) so probs@V consumes
exp(scores) directly -- no prob transposes. Row-sums come free from 64
ones-columns in an fp16 token-major V_aug (PSUM rows 64:128 hold r); sink is
one fp16 add on the evacuated tile, then reciprocal + per-head multiplies.
Masking is MULTIPLICATIVE post-exp (fp16, DVE 4x mode) -- QR is zero-padded
128 cols each side so every kb tile is a uniform N=256 matmul with no PSUM
garbage. V is projected feature-major then PE-transposed per 128-block.
The whole rope/attention elementwise pipeline runs in fp16. Heads are paired
(g0, g1) per Q tile host-side so QK needs no K half-swaps; head pairs share
one QK PSUM bank so exp/mask run on [128,512] tiles. Output is fp16;
host sums partials in fp32. Input DMAs are coalesced and ordered so compute
starts after ~3 chunks.
"""
import numpy as np

# ---- problem constants (hardcoded per contract) ----
B, S, E = 1, 2048, 2048
H, G, D = 32, 8, 64
SW = 128
ROPE_BASE = 10000.0
ORIG_CTX = 4096.0
YARN_SCALE = 2.0
BETA_FAST, BETA_SLOW = 32.0, 1.0
NEG = -1e30

# ---- sharding constants ----
NCORES = 8
TOK = 1152           # local kv tokens (9 blocks of 128)
NQ = 1024            # local query tokens (kv blocks 1..8)
NKB = TOK // 128     # 9 kv blocks
QH = 8               # q heads per core
KG = 2               # kv groups per core
NE = E // 128        # 16 e-chunks
SCALE = 1.0 / (D ** 0.5)

_compiled = None


def _build_bass():
    import concourse.bacc as bacc
    import concourse.tile as tile
    import concourse.mybir as mybir
    from concourse.masks import make_identity

    f32 = mybir.dt.float32
    fp16 = mybir.dt.float16
    Exp = mybir.ActivationFunctionType.Exp
    Ident = mybir.ActivationFunctionType.Identity

    nc = bacc.Bacc("TRN2", target_bir_lowering=False, debug=False,
                   num_devices=NCORES)

    xT = nc.dram_tensor("xT", [E, TOK], fp16, kind="ExternalInput").ap()
    wqT = nc.dram_tensor("wqT", [E, 512], fp16, kind="ExternalInput").ap()
    wkvT = nc.dram_tensor("wkvT", [E, 256], fp16, kind="ExternalInput").ap()
    bqD = nc.dram_tensor("bq", [128, 4], f32, kind="ExternalInput").ap()
    bkvD = nc.dram_tensor("bkv", [128, 2], f32, kind="ExternalInput").ap()
    woutT = nc.dram_tensor("woutT", [512, E], fp16, kind="ExternalInput").ap()
    cosQ = nc.dram_tensor("cosQ", [128, NQ], fp16, kind="ExternalInput").ap()
    sinQ = nc.dram_tensor("sinQ", [128, NQ], fp16, kind="ExternalInput").ap()
    cosK = nc.dram_tensor("cosK", [128, TOK], fp16, kind="ExternalInput").ap()
    sinK = nc.dram_tensor("sinK", [128, TOK], fp16, kind="ExternalInput").ap()
    maskD = nc.dram_tensor("maskM", [128, NKB, 256], fp16, kind="ExternalInput").ap()
    esD = nc.dram_tensor("esb", [128, QH * 128], fp16, kind="ExternalInput").ap()
    outT = nc.dram_tensor("outT", [E, NQ], fp16, kind="ExternalOutput").ap()

    xT_r = xT.rearrange("(a p) t -> p a t", p=128)        # [128, 16, TOK]
    wq_r = wqT.rearrange("(a p) f -> p a f", p=128)       # [128, 16, 512]
    wkv_r = wkvT.rearrange("(a p) f -> p a f", p=128)     # [128, 16, 256]
    wo_r = woutT.rearrange("(a p) e -> p a e", p=128)     # [128, 4, 2048]
    out_r = outT.rearrange("(a p) q -> p a q", p=128)     # [128, 16, 1024]

    with tile.TileContext(nc) as tc, \
         nc.allow_low_precision(reason="fp16 psum accum within error budget"):
        with tc.tile_pool(name="persist", bufs=1) as persist, \
             tc.tile_pool(name="work", bufs=4) as work, \
             tc.tile_pool(name="pexs", bufs=5) as pexs, \
             tc.tile_pool(name="pexm", bufs=12) as pexm, \
             tc.tile_pool(name="pnorm", bufs=6) as pnorm, \
             tc.tile_pool(name="pout", bufs=3) as pout, \
             tc.tile_pool(name="psBig", bufs=2, space="PSUM") as psBig, \
             tc.tile_pool(name="psQK", bufs=4, space="PSUM") as psQK, \
             tc.tile_pool(name="psPV", bufs=2, space="PSUM") as psPV:

            # ---------- persistent tiles ----------
            x_sb = persist.tile([128, NE, TOK], fp16)
            wq_sb = persist.tile([128, NE, 512], fp16)
            wkv_sb = persist.tile([128, NE, 256], fp16)
            wo_sb = persist.tile([128, 4, E], fp16)
            bq_sb = persist.tile([128, 4], f32)
            bkv_sb = persist.tile([128, 2], f32)
            # Q tables padded 128 cols at the start to share QR's col space
            cQ = persist.tile([128, NQ + 256], fp16)
            sQ = persist.tile([128, NQ + 256], fp16)
            cK = persist.tile([128, TOK], fp16)
            sK = persist.tile([128, TOK], fp16)
            mask_sb = persist.tile([128, NKB, 256], fp16)
            es_sb = persist.tile([128, QH * 128], fp16)
            identf = persist.tile([128, 128], mybir.dt.float32)
            ident = persist.tile([128, 128], fp16)

            # DMAs in dependency order (HWDGE drains in issue order)
            nc.sync.dma_start(bq_sb, bqD)
            nc.sync.dma_start(bkv_sb, bkvD)
            nc.sync.dma_start(wkv_sb[:, :, 0:128], wkv_r[:, :, 0:128])
            nc.sync.dma_start(x_sb[:, :, 0:384], xT_r[:, :, 0:384])
            nc.sync.dma_start(cK, cosK)
            nc.sync.dma_start(sK, sinK)
            nc.sync.dma_start(wq_sb[:, :, 0:128], wq_r[:, :, 0:128])
            nc.sync.dma_start(cQ[:, 128:128 + NQ], cosQ)
            nc.sync.dma_start(sQ[:, 128:128 + NQ], sinQ)
            nc.sync.dma_start(wkv_sb[:, :, 128:256], wkv_r[:, :, 128:256])
            for i in range(1, 4):
                nc.sync.dma_start(wq_sb[:, :, 128 * i:128 * (i + 1)],
                                  wq_r[:, :, 128 * i:128 * (i + 1)])
            nc.sync.dma_start(x_sb[:, :, 384:768], xT_r[:, :, 384:768])
            nc.sync.dma_start(mask_sb, maskD)
            nc.sync.dma_start(x_sb[:, :, 768:1152], xT_r[:, :, 768:1152])
            nc.sync.dma_start(es_sb, esD)
            nc.sync.dma_start(wo_sb, wo_r)

            make_identity(nc, identf)
            nc.vector.tensor_copy(ident, identf)

            # rope targets; QR padded 128 zero-cols each side for uniform QK
            QR = [persist.tile([128, NQ + 256], fp16, name=f"QR{i}")
                  for i in range(4)]
            for i in range(4):
                nc.gpsimd.memset(QR[i][:, 0:128], 0.0)
                nc.gpsimd.memset(QR[i][:, NQ + 128:NQ + 256], 0.0)
            KR = persist.tile([128, TOK], fp16)
            # token-major V + 64 ones-cols per group
            V_aug = persist.tile([128, NKB, 2, 128], fp16)
            nc.gpsimd.memset(V_aug[:, :, :, 64:128], 1.0)
            # normalized context: tile i rows = (head i | head i+4), fp16
            ctxT = [persist.tile([128, NQ], fp16, name=f"ctxT{i}")
                    for i in range(4)]

            def rope(src, cT, sT, dst, cs_, csd_):
                # rotate-half for two stacked 64-row heads
                n = cs_.stop - cs_.start
                r = work.tile([128, n], fp16, tag="rot", name="rot")
                nc.vector.tensor_copy(r[0:32, :], src[32:64, cs_])
                nc.vector.tensor_copy(r[32:64, :], src[0:32, cs_])
                nc.gpsimd.tensor_copy(r[64:96, :], src[96:128, cs_])
                nc.scalar.copy(r[96:128, :], src[64:96, cs_])
                a = work.tile([128, n], fp16, tag="a", name="a")
                nc.vector.tensor_mul(a, src[:, cs_], cT[:, csd_])
                nc.vector.tensor_mul(r, r, sT[:, csd_])
                nc.gpsimd.tensor_add(dst[:, csd_], a, r)

            KCH = ((0, 384), (384, 768), (768, 1152))

            def emit_k(t):
                lo, hi = KCH[t]
                cs = slice(lo, hi)
                ps = psBig.tile([128, hi - lo], f32, tag="big", name="psk",
                                padded_shape=[128, 512])
                for e in range(NE):
                    nc.tensor.matmul(ps, wkv_sb[:, e, 0:128], x_sb[:, e, cs],
                                     start=(e == 0), stop=(e == NE - 1))
                kf = work.tile([128, hi - lo], fp16, tag="kf", name="kf")
                nc.scalar.activation(out=kf, in_=ps, func=Ident,
                                     bias=bkv_sb[:, 0:1])
                rope(kf, cK, sK, KR, slice(0, hi - lo), cs)

            # Q chunks aligned to x DMA chunks; QR data cols offset +128
            QCH = ((0, 256), (256, 640), (640, 1024))

            def emit_q(t):
                qlo, qhi = QCH[t]
                for i in range(4):
                    ps = psBig.tile([128, qhi - qlo], f32, tag="big", name="psq",
                                    padded_shape=[128, 512])
                    for e in range(NE):
                        nc.tensor.matmul(ps, wq_sb[:, e, 128 * i:128 * (i + 1)],
                                         x_sb[:, e, 128 + qlo:128 + qhi],
                                         start=(e == 0), stop=(e == NE - 1))
                    qf = work.tile([128, qhi - qlo], fp16, tag="qf", name="qf")
                    nc.scalar.activation(out=qf, in_=ps, func=Ident,
                                         bias=bq_sb[:, i:i + 1])
                    rope(qf, cQ, sQ, QR[i], slice(0, qhi - qlo),
                         slice(128 + qlo, 128 + qhi))

            def emit_v(t):
                cs = slice(384 * t, 384 * (t + 1))
                ps = psBig.tile([128, 384], f32, tag="big", name="psv",
                                padded_shape=[128, 512])
                for e in range(NE):
                    nc.tensor.matmul(ps, wkv_sb[:, e, 128:256], x_sb[:, e, cs],
                                     start=(e == 0), stop=(e == NE - 1))
                vf = work.tile([128, 384], fp16, tag="vf", name="vf")
                nc.scalar.activation(out=vf, in_=ps, func=Ident,
                                     bias=bkv_sb[:, 1:2])
                for j in range(3):
                    kb = 3 * t + j
                    pt = psBig.tile([128, 128], fp16, tag="big", name="pt",
                                    padded_shape=[128, 1024])
                    nc.tensor.transpose(pt, vf[:, 128 * j:128 * (j + 1)], ident)
                    nc.vector.tensor_copy(V_aug[:, kb, 0, 0:64], pt[:, 0:64])
                    nc.vector.tensor_copy(V_aug[:, kb, 1, 0:64], pt[:, 64:128])

            expT = {}

            def emit_qk(kb):
                # scT[k, q] for q-window [128*kb, 128*kb+256); QR cols are
                # q+128 with 128 pad cols, so rhs cols = 128*kb : 128*kb+256
                qs = slice(128 * kb, 128 * kb + 256)
                for j in range(4):          # head pairs (2j, 2j+1)
                    sc = psQK.tile([128, 512], f32, tag="sc", name="sc")
                    for u in range(2):
                        h = 2 * j + u
                        i, g = h % 4, h // 4
                        hs = slice(64 * g, 64 * (g + 1))
                        nc.tensor.matmul(sc[:, 256 * u:256 * (u + 1)],
                                         KR[hs, 128 * kb:128 * (kb + 1)],
                                         QR[i][hs, qs],
                                         start=True, stop=True)
                    exs = pexs.tile([128, 512], fp16, tag="exs", name="exs")
                    nc.scalar.activation(out=exs, in_=sc, func=Exp)
                    exm = pexm.tile([128, 512], fp16, tag="exm", name="exm")
                    for u in range(2):
                        eng = nc.vector if (j + u) % 2 == 0 else nc.gpsimd
                        eng.tensor_mul(exm[:, 256 * u:256 * (u + 1)],
                                       exs[:, 256 * u:256 * (u + 1)],
                                       mask_sb[:, kb, :])
                    expT[(j, kb)] = exm

            def emit_pv(qb):
                for g in range(2):
                    pv = psPV.tile([128, 512], f32, tag="pv", name="pv")
                    for m in range(4):      # head g*4+m -> pair j, half u
                        h = 4 * g + m
                        j, u = h // 2, h % 2
                        nc.tensor.matmul(pv[:, 128 * m:128 * (m + 1)],
                                         V_aug[:, qb, g, :],
                                         expT[(j, qb)][:, 256 * u + 128:256 * u + 256],
                                         start=True, stop=False)
                        nc.tensor.matmul(pv[:, 128 * m:128 * (m + 1)],
                                         V_aug[:, qb + 1, g, :],
                                         expT[(j, qb + 1)][:, 256 * u:256 * u + 128],
                                         start=False, stop=True)
                    pvs = pnorm.tile([128, 512], fp16, tag="pvs", name="pvs")
                    nc.scalar.copy(pvs, pv)
                    # rows 64:128 hold r replicated; add exp(sink), recip
                    nc.vector.tensor_add(pvs[64:128, :], pvs[64:128, :],
                                         es_sb[64:128, 512 * g:512 * (g + 1)])
                    rsb = pnorm.tile([64, 512], fp16, tag="rsb", name="rsb")
                    nc.vector.reciprocal(rsb, pvs[64:128, :])
                    for m in range(4):
                        h = 4 * g + m
                        i = h % 4
                        nc.vector.tensor_mul(
                            ctxT[i][64 * g:64 * (g + 1), 128 * qb:128 * (qb + 1)],
                            pvs[0:64, 128 * m:128 * (m + 1)],
                            rsb[:, 128 * m:128 * (m + 1)])

            def emit_outproj(lo, hi):
                # two e-values packed per PSUM bank -> one evac per pair
                o_full = pout.tile([128, NE, 128], fp16, tag="o", name="o")
                o_sb = o_full[:, :, 0:hi - lo]
                for e2 in range(NE // 2):
                    ps = psBig.tile([128, 2, hi - lo], f32, tag="big",
                                    name="pso", padded_shape=[128, 2, 256])
                    for half in range(2):
                        e = 2 * e2 + half
                        for i in range(4):
                            nc.tensor.matmul(
                                ps[:, half, :],
                                wo_sb[:, i, 128 * e:128 * (e + 1)],
                                ctxT[i][:, lo:hi],
                                start=(i == 0), stop=(i == 3))
                    if e2 % 2 == 0:
                        nc.scalar.copy(o_sb[:, 2 * e2:2 * e2 + 2, :], ps)
                    else:
                        nc.vector.tensor_copy(o_sb[:, 2 * e2:2 * e2 + 2, :], ps)
                    if e2 == 3 and hi == NQ:
                        nc.sync.dma_start(out_r[:, 0:8, lo:hi], o_sb[:, 0:8, :])
                if hi == NQ:
                    nc.sync.dma_start(out_r[:, 8:16, lo:hi], o_sb[:, 8:16, :])
                else:
                    nc.sync.dma_start(out_r[:, :, lo:hi], o_sb)

            # ---------- interleaved schedule ----------
            # within an attention group, out-proj MMs sit between qk(kb) and
            # pv(kb-1) so PE covers the exp/mask latency of kb
            emit_k(0)
            emit_q(0)
            emit_v(0)
            emit_k(1)
            emit_k(2)
            emit_qk(0)
            emit_q(1)
            emit_qk(1); emit_pv(0)
            emit_q(2)
            emit_v(1)
            emit_qk(2); emit_outproj(0, 128); emit_pv(1)
            emit_qk(3); emit_outproj(128, 256); emit_pv(2)
            emit_qk(4); emit_outproj(256, 384); emit_pv(3)
            emit_v(2)
            emit_qk(5); emit_outproj(384, 512); emit_pv(4)
            emit_qk(6); emit_outproj(512, 640); emit_pv(5)
            emit_qk(7); emit_outproj(640, 768); emit_pv(6)
            emit_qk(8); emit_outproj(768, 896); emit_pv(7)
            emit_outproj(896, 1024)

    nc.compile()
    return nc


# ---------------- host-side prep ----------------

def _rope_tables(position_ids, gstart, lo, n):
    pos = np.zeros(n, dtype=np.float32)
    idx = gstart + lo + np.arange(n)
    valid = (idx >= 0) & (idx < S)
    pos[valid] = position_ids[0, idx[valid]].astype(np.float32)
    freqs = (1.0 / ROPE_BASE ** (np.arange(0, D, 2, dtype=np.float32) / D)).astype(np.float32)
    wave_len = 2.0 * np.pi / freqs
    t = np.clip((wave_len - ORIG_CTX / BETA_FAST)
                / (ORIG_CTX / BETA_SLOW - ORIG_CTX / BETA_FAST), 0.0, 1.0)
    eff = freqs * (1.0 - t) + (freqs / YARN_SCALE) * t
    conc = 0.1 * np.log(np.float32(YARN_SCALE)) + 1.0
    ang = pos[:, None] * eff[None, :] * conc
    sin = np.sin(ang).astype(np.float32).T    # [32, n]
    cos = np.cos(ang).astype(np.float32).T
    cosT = np.concatenate([cos, cos], axis=0)      # [64, n]
    sinS = np.concatenate([-sin, sin], axis=0)
    cos2 = np.concatenate([cosT, cosT], axis=0)    # [128, n]
    sinS2 = np.concatenate([sinS, sinS], axis=0)
    return np.ascontiguousarray(cos2), np.ascontiguousarray(sinS2)


def _build_maskM(attn_mask, s, gstart):
    # multiplicative mask exp(a): [p, kb, c]; key k = 128*kb+p, q = 128*kb+c
    kb = np.arange(NKB)[None, :, None]
    p = np.arange(128)[:, None, None]
    c = np.arange(256)[None, None, :]
    lk = 128 * kb + p
    lq = 128 * kb + c
    gk = gstart + lk
    gq = gstart + lq
    gk_b, gq_b = np.broadcast_arrays(gk, gq)
    valid = ((lq >= 128) & (lq < TOK) & (gk_b >= 0)
             & (gk_b <= gq_b) & (gk_b > gq_b - SW))
    add = np.where(
        valid,
        np.maximum(attn_mask[0, 0, np.clip(gq_b, 0, S - 1),
                             np.clip(gk_b, 0, S - 1)], NEG),
        NEG)
    return np.ascontiguousarray(np.exp(add).astype(np.float32))


def _prep_core(c, x, position_ids, attn_mask, Wqkv, bqkv, Wout, sinks, xT_full):
    s, hg = c // 4, c % 4
    gstart = 1024 * s - 128
    xTc = np.zeros((E, TOK), dtype=np.float32)
    lo = max(0, gstart)
    xTc[:, lo - gstart:TOK] = xT_full[:, lo:gstart + TOK]
    # head pairing: Q tile i rows = (head 8*hg+i | head 8*hg+4+i)
    qheads = [(8 * hg + i, 8 * hg + 4 + i) for i in range(4)]
    qrows = np.concatenate(
        [np.concatenate([np.arange(64 * a, 64 * a + 64),
                         np.arange(64 * b, 64 * b + 64)])
         for (a, b) in qheads])
    krows = np.arange(H * D + 128 * hg, H * D + 128 * hg + 128)
    vrows = np.arange((H + G) * D + 128 * hg, (H + G) * D + 128 * hg + 128)
    kvrows = np.concatenate([krows, vrows])
    f16 = np.float16
    wqT = np.ascontiguousarray(Wqkv[qrows].T.astype(f16))
    wkvT = np.ascontiguousarray(Wqkv[kvrows].T.astype(f16))
    bq = np.ascontiguousarray(bqkv[qrows].reshape(4, 128).T.astype(np.float32))
    bkv = np.ascontiguousarray(bqkv[kvrows].reshape(2, 128).T.astype(np.float32))
    woutT = np.ascontiguousarray(Wout[:, qrows].T.astype(f16))
    cosQ, sinQ = _rope_tables(position_ids, gstart, 128, NQ)
    cosK, sinK = _rope_tables(position_ids, gstart, 0, TOK)
    maskM = _build_maskM(attn_mask, s, gstart)
    es = np.exp(sinks[0, 8 * hg:8 * hg + QH, 0, 0].astype(np.float32))
    es_bc = np.ascontiguousarray(
        np.broadcast_to(np.repeat(es, 128)[None, :], (128, QH * 128))
    ).astype(f16)
    return {
        "xT": np.ascontiguousarray(xTc.astype(f16)),
        "wqT": wqT, "wkvT": wkvT,
        "bq": bq, "bkv": bkv,
        "woutT": woutT,
        "cosQ": np.ascontiguousarray((SCALE * cosQ).astype(f16)),
        "sinQ": np.ascontiguousarray((SCALE * sinQ).astype(f16)),
        "cosK": np.ascontiguousarray(cosK.astype(f16)),
        "sinK": np.ascontiguousarray(sinK.astype(f16)),
        "maskM": np.ascontiguousarray(maskM.astype(f16)),
        "esb": es_bc,
    }


def kernel(x, position_ids, attn_mask, Wqkv, bqkv, Wout, bout, sinks):
    global _compiled
    from concourse.bass_utils import run_bass_kernel_spmd

    x = np.asarray(x, dtype=np.float32)
    position_ids = np.asarray(position_ids)
    attn_mask = np.asarray(attn_mask, dtype=np.float32)
    Wqkv = np.asarray(Wqkv, dtype=np.float32)
    bqkv = np.asarray(bqkv, dtype=np.float32)
    Wout = np.asarray(Wout, dtype=np.float32)
    bout = np.asarray(bout, dtype=np.float32)
    sinks = np.asarray(sinks, dtype=np.float32)

    if _compiled is None:
        _compiled = _build_bass()
    nc = _compiled

    xT_full = np.ascontiguousarray(x[0].T)
    in_maps = [
        _prep_core(c, x, position_ids, attn_mask, Wqkv, bqkv, Wout, sinks, xT_full)
        for c in range(NCORES)
    ]
    res = run_bass_kernel_spmd(nc, in_maps, list(range(NCORES)))

    out = np.empty((S, E), dtype=np.float32)
    for s in range(2):
        acc = res.results[4 * s]["outT"].astype(np.float32)
        for h in range(1, 4):
            acc = acc + res.results[4 * s + h]["outT"].astype(np.float32)
        out[1024 * s:1024 * (s + 1)] = acc.T
    out += bout[None, :]
    return out[None]
